# revision 1
# baseline (speedup 1.0000x reference)
"""Additive (Bahdanau) attention on 8 trn2 NeuronCores — sine-expansion kernel.

Math per batch element b (one core each):
  logits[q,k] = sum_a w_a * tanh(x_qa + y_ka),  x = query@Wq^T, y = key@Wk^T + bias
  attn = softmax_k(logits);  out = attn @ value

Instead of materializing the [TQ,TK,A] tanh tensor (ScalarE-bound, ~280us),
approximate tanh(z) ~ alpha*z + sum_h c_h sin(w_h z) with two harmonic ladders
w = a*{1..5} u b*{1..8}.  Each sine term factors:
  sin(w(x+y)) = sin(wx)cos(wy) + cos(wx)sin(wy)
so logits becomes 26 rank-128 fp16 matmuls (contraction over a) on the PE,
plus a rank-1 f32 term for the (softmax-relevant) linear-in-y part.

Harmonics sin(m*w0*t), cos(m*w0*t) are generated with the Chebyshev three-term
recurrence on the DVE in fp16 (4x mode), seeded from ACT Sin at half/quarter
angle (HW Sin is only accurate for |arg| <= 3.0 — no range reduction).
All four trig arrays per harmonic live in one [128, 2048] tile so each
recurrence step is 2 wide DVE ops.  The fit coefficients are folded into the
q-side features (one tensor_scalar per harmonic); softmax epilogue matches the
previous kernel (Exp with accumulated row sums, normalization folded into the
psum->sbuf copies as a per-partition ACT scale).
"""

import numpy as np

import concourse.bass as bass
import concourse.tile as tile
from concourse import bacc, mybir
from concourse.bass_utils import run_bass_kernel_spmd

F32 = mybir.dt.float32
FP16 = mybir.dt.float16
AF = mybir.ActivationFunctionType
ALU = mybir.AluOpType

B, TQ, TK, DQ, DK, DV, A = 8, 512, 512, 512, 512, 512, 128
N_CORES = 8

# two-ladder sine fit of tanh on [-10.4, 10.4] (see build notes)
WA, WB = 0.41, 0.62
MA, MB = 5, 6
M_TOT = MA + MB
HALF_PI = float(np.pi / 2)

_CACHE = {}


def _fit_params():
    if "fit" in _CACHE:
        return _CACHE["fit"]
    zg = np.linspace(-10.40, 10.40, 6001)
    t = np.tanh(zg)
    freqs = np.array(
        list(WA * np.arange(1, MA + 1)) + list(WB * np.arange(1, MB + 1)))
    Amat = np.stack([np.sin(w * zg) for w in freqs] + [zg], 1)
    coef, *_ = np.linalg.lstsq(Amat, t, rcond=None)
    _CACHE["fit"] = (coef[:-1].astype(np.float64), float(coef[-1]))
    return _CACHE["fit"]


def build_nc():
    nc = bacc.Bacc(None, target_bir_lowering=False, debug=False)

    qT = nc.declare_dram_parameter("qT", [DQ, TQ], FP16, isOutput=False)
    kT = nc.declare_dram_parameter("kT", [DK, TK], FP16, isOutput=False)
    val = nc.declare_dram_parameter("value", [TK, DV], FP16, isOutput=False)
    WqT = nc.declare_dram_parameter("WqT", [DQ, A], FP16, isOutput=False)
    WkT = nc.declare_dram_parameter("WkT", [DK, A], FP16, isOutput=False)
    # consts f32: col0 = additive bias b_a, col1 = 0, col2 = pi/2
    consts = nc.declare_dram_parameter("consts", [A, 4], F32, isOutput=False)
    # wc f32 [128, M_TOT]: w_a * c_h
    wc_d = nc.declare_dram_parameter("wc", [A, M_TOT], F32, isOutput=False)
    # uvec fp16 [128, 4]: alpha * (w @ Wk) reshaped (d = c*128 + p)
    uvec_d = nc.declare_dram_parameter("uvec", [128, DK // 128], FP16,
                                       isOutput=False)
    ones_d = nc.declare_dram_parameter("ones_row", [1, 128], F32,
                                       isOutput=False)
    ident_d = nc.declare_dram_parameter("ident", [128, 128], FP16,
                                        isOutput=False)
    attn_out = nc.declare_dram_parameter("attn", [TQ, TK], F32, isOutput=True)
    out_out = nc.declare_dram_parameter("out", [TQ, DV], F32, isOutput=True)

    with tile.TileContext(nc) as tc:
        with (
            tc.tile_pool(name="pers", bufs=1) as pers,
            tc.tile_pool(name="tmp", bufs=2) as tmp_pool,
            tc.tile_pool(name="epi", bufs=4) as epi_pool,
            tc.tile_pool(name="small", bufs=4) as small_pool,
            tc.tile_pool(name="plog", bufs=4, space="PSUM") as plog_pool,
            tc.tile_pool(name="ptr", bufs=1, space="PSUM") as ptr_pool,
            tc.tile_pool(name="pout", bufs=1, space="PSUM") as pout_pool,
        ):
            # ---- persistent tiles ----
            cst = pers.tile([128, 4], F32, tag="cst0")
            wc = pers.tile([128, M_TOT], F32, tag="wc1")
            ident = pers.tile([128, 128], FP16, tag="ident2")
            ones_r = pers.tile([1, 128], F32, tag="ones_r3")
            uvec = pers.tile([128, DK // 128], FP16, tag="uvec4")
            WkT_sb = pers.tile([128, DK // 128, A], FP16, tag="WkT_sb5")
            WqT_sb = pers.tile([128, DQ // 128, A], FP16, tag="WqT_sb6")
            kT_sb = pers.tile([128, DK // 128, TK], FP16, tag="kT_sb7")
            qT_sb = pers.tile([128, DQ // 128, TQ], FP16, tag="qT_sb8")
            value_sb = pers.tile([128, TK // 128, DV], FP16, tag="value_sb9")
            k_pb = pers.tile([128, TK], F32, tag="k_pb10")
            q_pT = pers.tile([128, TQ], F32, tag="q_pT11")
            rk_sb = pers.tile([1, TK], F32, tag="rk_sb12")

            # ---- input DMA: k-path first (SP), q-path on Pool sequencer,
            # tiny consts afterwards, value last ----
            nc.sync.dma_start(
                out=WkT_sb[:], in_=WkT.rearrange("(c p) a -> p c a", p=128))
            kT_re = kT.rearrange("(c p) t -> p c t", p=128)
            for c in range(4):
                nc.sync.dma_start(out=kT_sb[:, c:c + 1, :],
                                  in_=kT_re[:, c:c + 1, :])
            nc.gpsimd.dma_start(
                out=WqT_sb[:], in_=WqT.rearrange("(c p) a -> p c a", p=128))
            qT_re = qT.rearrange("(c p) t -> p c t", p=128)
            for c in range(4):
                nc.gpsimd.dma_start(out=qT_sb[:, c:c + 1, :],
                                    in_=qT_re[:, c:c + 1, :])
            nc.sync.dma_start(out=cst[:], in_=consts[:, :])
            nc.sync.dma_start(out=wc[:], in_=wc_d[:, :])
            nc.sync.dma_start(out=ident[:], in_=ident_d[:, :])
            nc.sync.dma_start(out=ones_r[:], in_=ones_d[:, :])
            nc.sync.dma_start(out=uvec[:], in_=uvec_d[:, :])
            val_re = val.rearrange("(c p) d -> p c d", p=128)
            nc.sync.dma_start(out=value_sb[:, 0:2, :], in_=val_re[:, 0:2, :])
            nc.sync.dma_start(out=value_sb[:, 2:4, :], in_=val_re[:, 2:4, :])

            zb = cst[:, 1:2]
            halfpi = cst[:, 2:3]

            # ---- projections (fp16 matmuls, f32 psum) ----
            k_ps = plog_pool.tile([128, TK], F32, tag="plog")
            for c in range(DK // 128):
                nc.tensor.matmul(k_ps[:], WkT_sb[:, c, :], kT_sb[:, c, :],
                                 start=(c == 0), stop=(c == DK // 128 - 1))
            nc.scalar.activation(k_pb[:], k_ps[:], AF.Identity,
                                 bias=cst[:, 0:1], scale=1.0)

            q_ps = plog_pool.tile([128, TQ], F32, tag="plog")
            for c in range(DQ // 128):
                nc.tensor.matmul(q_ps[:], WqT_sb[:, c, :], qT_sb[:, c, :],
                                 start=(c == 0), stop=(c == DQ // 128 - 1))
            nc.scalar.copy(q_pT[:], q_ps[:])

            # rk = alpha * (w @ Wk) @ kT  (rank-1 softmax-relevant linear term)
            rk_ps = ptr_pool.tile([1, TK], F32)
            for c in range(DK // 128):
                nc.tensor.matmul(rk_ps[:], uvec[:, c:c + 1], kT_sb[:, c, :],
                                 start=(c == 0), stop=(c == DK // 128 - 1))
            nc.scalar.copy(rk_sb[:], rk_ps[:])

            # ---- seeds: ACT Sin at half/quarter angle, fp16 out ----
            # per side: shA = sin(a/2 t), chA = sin(a/2 t + pi/2),
            #           qhB = sin(b/4 t), shB = sin(b/2 t)
            seeds = {}
            for side, src in (("y", k_pb), ("x", q_pT)):
                shA = pers.tile([128, 512], FP16, tag=f"shA_{side}")
                chA = pers.tile([128, 512], FP16, tag=f"chA_{side}")
                qhB = pers.tile([128, 512], FP16, tag=f"qhB_{side}")
                shB = pers.tile([128, 512], FP16, tag=f"shB_{side}")
                nc.scalar.activation(shA[:], src[:], AF.Sin, bias=zb,
                                     scale=WA / 2)
                nc.scalar.activation(chA[:], src[:], AF.Sin, bias=halfpi,
                                     scale=WA / 2)
                nc.scalar.activation(qhB[:], src[:], AF.Sin, bias=zb,
                                     scale=WB / 4)
                nc.scalar.activation(shB[:], src[:], AF.Sin, bias=zb,
                                     scale=WB / 2)
                seeds[side] = (shA, chA, qhB, shB)

            # level tiles: [s_x | c_x | s_y | c_y], harmonic m of each family
            levA = [pers.tile([128, 2048], FP16, name="levA", tag=f"levA{i}") for i in range(MA)]
            levB = [pers.tile([128, 2048], FP16, name="levB", tag=f"levB{i}") for i in range(MB)]
            lv0 = pers.tile([128, 2048], FP16, tag="lv017")
            mulA = pers.tile([128, 2048], FP16, tag="mulA18")
            mulB = pers.tile([128, 2048], FP16, tag="mulB19")

            SX, CX, SY, CY = (slice(0, 512), slice(512, 1024),
                              slice(1024, 1536), slice(1536, 2048))

            # m=0 level: [0 | 1 | 0 | 1]
            nc.gpsimd.memset(lv0[:, SX], 0.0)
            nc.gpsimd.memset(lv0[:, CX], 1.0)
            nc.gpsimd.memset(lv0[:, SY], 0.0)
            nc.gpsimd.memset(lv0[:, CY], 1.0)

            # seeds -> first-harmonic levels (y side first; k_pb lands first)
            for side, ssl, csl in (("y", SY, CY), ("x", SX, CX)):
                shA, chA, qhB, shB = seeds[side]
                # family A: s1 = 2*shA*chA ; c1 = 1 - 2*shA^2
                nc.vector.scalar_tensor_tensor(
                    levA[0][:, ssl], shA[:], 2.0, chA[:], ALU.mult, ALU.mult)
                uA = tmp_pool.tile([128, 512], FP16)
                nc.vector.tensor_tensor(uA[:], shA[:], shA[:], ALU.mult)
                nc.vector.tensor_scalar(
                    levA[0][:, csl], uA[:], -2.0, 1.0, ALU.mult, ALU.add)
                # family B: chB = 1-2*qhB^2 ; s1 = 2*shB*chB ; c1 = 1-2*shB^2
                uB = tmp_pool.tile([128, 512], FP16)
                nc.vector.tensor_tensor(uB[:], qhB[:], qhB[:], ALU.mult)
                chB = tmp_pool.tile([128, 512], FP16)
                nc.vector.tensor_scalar(
                    chB[:], uB[:], -2.0, 1.0, ALU.mult, ALU.add)
                nc.vector.scalar_tensor_tensor(
                    levB[0][:, ssl], shB[:], 2.0, chB[:], ALU.mult, ALU.mult)
                uB2 = tmp_pool.tile([128, 512], FP16)
                nc.vector.tensor_tensor(uB2[:], shB[:], shB[:], ALU.mult)
                nc.vector.tensor_scalar(
                    levB[0][:, csl], uB2[:], -2.0, 1.0, ALU.mult, ALU.add)

            # multipliers: [2c1 | 2c1 | 2c1y | 2c1y] per family
            for mul, lev1 in ((mulA, levA[0]), (mulB, levB[0])):
                for dsl, ssl2 in ((SX, CX), (CX, CX), (SY, CY), (CY, CY)):
                    nc.scalar.activation(mul[:, dsl], lev1[:, ssl2], AF.Copy,
                                         bias=0.0, scale=2.0)

            # ---- logits accumulation helpers ----
            coeffs, alpha = _fit_params()
            plogs = [plog_pool.tile([128, TK], F32, name="plog_g", tag="plog")
                     for _ in range(4)]
            # rank-1 linear term opens each accumulation group (f32 matmul)
            for g in range(4):
                nc.tensor.matmul(plogs[g][:], ones_r[:], rk_sb[:],
                                 start=True, stop=False)

            qws = []

            def emit_products(lev, h, last):
                qw = pers.tile([128, 1024], FP16, tag=f"qw{h}")
                qws.append(qw)
                nc.scalar.activation(qw[:], lev[:, 0:1024], AF.Copy,
                                     bias=0.0, scale=wc[:, h:h + 1])
                for g in range(4):
                    qsl = slice(g * 128, (g + 1) * 128)
                    csl = slice(512 + g * 128, 512 + (g + 1) * 128)
                    nc.tensor.matmul(plogs[g][:], qw[:, qsl], lev[:, CY],
                                     start=False, stop=False)
                    nc.tensor.matmul(plogs[g][:], qw[:, csl], lev[:, SY],
                                     start=False, stop=last)

            def advance(lev_list, mul, m):
                # lev[m] = mul * lev[m-1] - lev[m-2]
                prev = lev_list[m - 1]
                prev2 = lv0 if m == 1 else lev_list[m - 2]
                t = tmp_pool.tile([128, 2048], FP16)
                nc.vector.tensor_tensor(t[:], mul[:], prev[:], ALU.mult)
                nc.vector.tensor_tensor(lev_list[m][:], t[:], prev2[:],
                                        ALU.subtract)

            # family A products + recurrence
            emit_products(levA[0], 0, last=False)
            for m in range(1, MA):
                advance(levA, mulA, m)
                emit_products(levA[m], m, last=False)
            emit_products(levB[0], MA, last=False)
            for m in range(1, MB):
                advance(levB, mulB, m)
                emit_products(levB[m], MA + m, last=(m == MB - 1))

            # ---- epilogue per 128-query group ----
            for g in range(4):
                qbase = g * 128
                expt = epi_pool.tile([128, TK], FP16)
                rowsum = small_pool.tile([128, 1], F32)
                nc.scalar.activation(expt[:], plogs[g][:], AF.Exp, bias=zb,
                                     accum_out=rowsum[:, :])
                recip = small_pool.tile([128, 1], F32)
                nc.vector.reciprocal(recip[:, :], rowsum[:, :])
                attn_sb = epi_pool.tile([128, TK], F32)
                nc.vector.tensor_scalar_mul(attn_sb[:], expt[:],
                                            recip[:, 0:1])
                for s in range(4):
                    sl = slice(s * 128, (s + 1) * 128)
                    nc.gpsimd.dma_start(out=attn_out[qbase:qbase + 128, sl],
                                        in_=attn_sb[:, sl])
                out_ps = pout_pool.tile([128, DV], F32)
                for c in range(TK // 128):
                    tp = ptr_pool.tile([128, 128], FP16)
                    nc.tensor.transpose(tp[:], expt[:, c * 128:(c + 1) * 128],
                                        ident[:])
                    attnT = epi_pool.tile([128, 128], FP16)
                    nc.vector.tensor_copy(attnT[:], tp[:])
                    nc.tensor.matmul(out_ps[:], attnT[:], value_sb[:, c, :],
                                     start=(c == 0), stop=(c == TK // 128 - 1))
                out_sb = epi_pool.tile([128, DV], F32)
                nc.scalar.activation(out_sb[:], out_ps[:], AF.Copy, bias=0.0,
                                     scale=recip[:, 0:1])
                for s in range(4):
                    sl = slice(s * 128, (s + 1) * 128)
                    nc.sync.dma_start(out=out_out[qbase:qbase + 128, sl],
                                      in_=out_sb[:, sl])

    nc.compile()
    return nc


def _get_nc():
    if "nc" not in _CACHE:
        _CACHE["nc"] = build_nc()
    return _CACHE["nc"]


def make_in_maps(query, key, value, Wq, Wk, bias, w_w, **_):
    coeffs, alpha = _fit_params()
    w = np.asarray(w_w, dtype=np.float64).reshape(A)
    WqT = np.ascontiguousarray(Wq.T).astype(np.float16)
    WkT = np.ascontiguousarray(Wk.T).astype(np.float16)
    consts = np.zeros((A, 4), dtype=np.float32)
    consts[:, 0] = np.asarray(bias, dtype=np.float32).reshape(A)
    consts[:, 2] = HALF_PI
    wc = (w[:, None] * coeffs[None, :]).astype(np.float32)
    u = (alpha * (w @ np.asarray(Wk, dtype=np.float64)))  # [DK]
    uvec = u.reshape(DK // 128, 128).T.astype(np.float16)
    ones_row = np.ones((1, 128), dtype=np.float32)
    ident = np.eye(128, dtype=np.float16)
    in_maps = []
    for b in range(B):
        in_maps.append({
            "qT": np.ascontiguousarray(query[b].T).astype(np.float16),
            "kT": np.ascontiguousarray(key[b].T).astype(np.float16),
            "value": np.ascontiguousarray(value[b]).astype(np.float16),
            "WqT": WqT,
            "WkT": WkT,
            "consts": consts,
            "wc": wc,
            "uvec": uvec,
            "ones_row": ones_row,
            "ident": ident,
        })
    return in_maps


def run(inputs, trace=False, **kwargs):
    nc = _get_nc()
    in_maps = make_in_maps(**{k: np.asarray(v) for k, v in inputs.items()})
    res = run_bass_kernel_spmd(
        nc, in_maps, list(range(N_CORES)), trace=trace, **kwargs
    )
    output = np.stack([res.results[b]["out"] for b in range(B)])
    attn = np.stack([res.results[b]["attn"] for b in range(B)])
    return (output, attn), res


def kernel(**inputs):
    (output, attn), _ = run(inputs)
    return output, attn



# revision 12
# speedup vs baseline: 1.0881x; 1.0881x over previous
"""Additive (Bahdanau) attention on 8 trn2 NeuronCores — flipped sine-expansion.

Math per batch element b (one core each):
  logits[q,k] = sum_a w_a * tanh(x_qa + y_ka),  x = query@Wq^T, y = key@Wk^T + bias
  attn = softmax_k(logits);  out = attn @ value

tanh(z) ~ alpha*z + sum_h c_h sin(w_h z) with frequency set
  WA*{1,2,4,8} u WB*{1,2,3,4}  (fit on [-8.6, 8.6]).
Each sine factors sin(w(x+y)) = sin(wx)cos(wy)+cos(wx)sin(wy), so logits are
16 rank-128 fp16 matmuls accumulated in PSUM.  This version computes logits
TRANSPOSED ([k, q] with k on PSUM partitions):
  - the linear-in-y term alpha*(w@Wk)@kT becomes a per-partition Exp bias
    (no rank-1 PSUM opens); the per-q linear term cancels in softmax
  - no PE transposes in the out = attn@value epilogue (exp tiles are
    directly the stationary operand)
  - attn is written transposed/unnormalized-free as fp16; host transposes
Harmonics are built with fp16 DVE double-angle steps (s2=2sc, c2=1-2s^2,
tensor_scalar hits 4x mode) plus one Chebyshev step for b3; seeds come from
ACT Sin at half/quarter angle (|arg| <= 3).  Per-harmonic scale w_a*c_h is
applied to the y-side (stationary) slices as scaled copies on ScalarE/DVE.
"""

import numpy as np

import concourse.bass as bass
import concourse.tile as tile
from concourse import bacc, mybir
from concourse.bass_utils import run_bass_kernel_spmd

F32 = mybir.dt.float32
FP16 = mybir.dt.float16
AF = mybir.ActivationFunctionType
ALU = mybir.AluOpType

B, TQ, TK, DQ, DK, DV, A = 8, 512, 512, 512, 512, 512, 128
N_CORES = 8

WA, WB = 0.36, 0.55
RFIT = 8.6
HKEYS = ["a1", "a2", "a4", "a8", "b1", "b2", "b3", "b4"]
FREQS = [WA, 2 * WA, 4 * WA, 8 * WA, WB, 2 * WB, 3 * WB, 4 * WB]
HALF_PI = float(np.pi / 2)

# quadrant slices of a level tile [s_x | s_y | c_x | c_y]
SX, SY, CX, CY = (slice(0, 512), slice(512, 1024),
                  slice(1024, 1536), slice(1536, 2048))
SH, CH = slice(0, 1024), slice(1024, 2048)  # s-half, c-half

_CACHE = {}


def _fit_params():
    if "fit" in _CACHE:
        return _CACHE["fit"]
    zg = np.linspace(-RFIT, RFIT, 6001)
    t = np.tanh(zg)
    Amat = np.stack([np.sin(w * zg) for w in FREQS] + [zg], 1)
    coef, *_ = np.linalg.lstsq(Amat, t, rcond=None)
    _CACHE["fit"] = (coef[:-1].astype(np.float64), float(coef[-1]))
    return _CACHE["fit"]


def build_nc():
    nc = bacc.Bacc(None, target_bir_lowering=False, debug=False)

    qT = nc.declare_dram_parameter("qT", [DQ, TQ], FP16, isOutput=False)
    kT = nc.declare_dram_parameter("kT", [DK, TK], FP16, isOutput=False)
    val = nc.declare_dram_parameter("value", [TK, DV], FP16, isOutput=False)
    WqT = nc.declare_dram_parameter("WqT", [DQ, A], FP16, isOutput=False)
    WkT = nc.declare_dram_parameter("WkT", [DK, A], FP16, isOutput=False)
    # f32 consts: c0=WA/2*b, c1=WA/2*b+pi/2, c2=WB/4*b, c3=WB/2*b, c4=pi/2
    cst_d = nc.declare_dram_parameter("cst", [A, 8], F32, isOutput=False)
    # fp16 consts: c0=1.0, c1=alpha*w_a
    cst16_d = nc.declare_dram_parameter("cst16", [A, 2], FP16, isOutput=False)
    wc_d = nc.declare_dram_parameter("wc", [A, 8], F32, isOutput=False)
    onesR_d = nc.declare_dram_parameter("onesR", [1, 128], FP16, isOutput=False)
    ident_d = nc.declare_dram_parameter("ident", [128, 128], FP16,
                                        isOutput=False)
    attnT_o = nc.declare_dram_parameter("attnT", [TK, TQ], FP16, isOutput=True)
    outN_o = nc.declare_dram_parameter("outN", [TQ, DV], FP16, isOutput=True)

    with tile.TileContext(nc) as tc:
        with (
            tc.tile_pool(name="pers", bufs=1) as pers,
            tc.tile_pool(name="tmp", bufs=3) as tmp_pool,
            tc.tile_pool(name="t2k", bufs=1) as t2k_pool,
            tc.tile_pool(name="scy", bufs=3) as scy_pool,
            tc.tile_pool(name="epi", bufs=8) as epi_pool,
            tc.tile_pool(name="small", bufs=6) as small_pool,
            tc.tile_pool(name="psA", bufs=4, space="PSUM") as psA,
            tc.tile_pool(name="pout", bufs=4, space="PSUM") as pout_pool,
        ):
            # ---- persistent tiles ----
            cst = pers.tile([128, 8], F32, tag="cst")
            cst16 = pers.tile([128, 2], FP16, tag="cst16")
            wc = pers.tile([128, 8], F32, tag="wc")
            onesR = pers.tile([1, 128], FP16, tag="onesR")
            ident = pers.tile([128, 128], FP16, tag="ident")
            WkT_sb = pers.tile([128, DK // 128, A], FP16, tag="WkT_sb")
            WqT_sb = pers.tile([128, DQ // 128, A], FP16, tag="WqT_sb")
            kT_sb = pers.tile([128, DK // 128, TK], FP16, tag="kT_sb")
            qT_sb = pers.tile([128, DQ // 128, TQ], FP16, tag="qT_sb")
            value_sb = pers.tile([128, TK // 128, DV], FP16, tag="value_sb")
            y16 = pers.tile([128, TK], FP16, tag="y16")
            rk_sb = pers.tile([128, 4], F32, tag="rk_sb")
            # seeds [x | y]
            shA = pers.tile([128, 1024], FP16, tag="shA")
            chA = pers.tile([128, 1024], FP16, tag="chA")
            qhB = pers.tile([128, 1024], FP16, tag="qhB")
            shB = pers.tile([128, 1024], FP16, tag="shB")
            chB = pers.tile([128, 1024], FP16, tag="chB")
            lev = {h: pers.tile([128, 2048], FP16, name=f"lev_{h}", tag=f"lev_{h}")
                   for h in HKEYS}
            mulB = pers.tile([128, 2048], FP16, tag="mulB")
            exp_t = [pers.tile([128, TQ], FP16, name=f"exp{g}", tag=f"exp{g}")
                     for g in range(4)]

            # ---- input DMA (spread issue across sequencers/queues) ----
            kT_re = kT.rearrange("(c p) t -> p c t", p=128)
            qT_re = qT.rearrange("(c p) t -> p c t", p=128)
            val_re = val.rearrange("(c p) d -> p c d", p=128)
            nc.sync.dma_start(
                out=WkT_sb[:], in_=WkT.rearrange("(c p) a -> p c a", p=128))
            for c in range(2):
                nc.sync.dma_start(out=kT_sb[:, c:c + 1, :],
                                  in_=kT_re[:, c:c + 1, :])
            for c in range(2, 4):
                nc.scalar.dma_start(out=kT_sb[:, c:c + 1, :],
                                    in_=kT_re[:, c:c + 1, :])
            nc.sync.dma_start(out=cst[:], in_=cst_d[:, :])
            nc.sync.dma_start(out=wc[:], in_=wc_d[:, :])
            nc.scalar.dma_start(out=cst16[:], in_=cst16_d[:, :])
            nc.gpsimd.dma_start(
                out=WqT_sb[:], in_=WqT.rearrange("(c p) a -> p c a", p=128))
            for c in range(4):
                nc.gpsimd.dma_start(out=qT_sb[:, c:c + 1, :],
                                    in_=qT_re[:, c:c + 1, :])
            nc.gpsimd.dma_start(out=onesR[:], in_=onesR_d[:, :])
            nc.scalar.dma_start(out=ident[:], in_=ident_d[:, :])
            nc.sync.dma_start(out=value_sb[:, 0:2, :], in_=val_re[:, 0:2, :])
            nc.gpsimd.dma_start(out=value_sb[:, 2:4, :], in_=val_re[:, 2:4, :])

            # ---- k projection + y seeds ----
            k_ps = psA.tile([128, TK], F32, tag="psA")
            for c in range(DK // 128):
                nc.tensor.matmul(k_ps[:], WkT_sb[:, c, :], kT_sb[:, c, :],
                                 start=(c == 0), stop=(c == DK // 128 - 1))
            nc.vector.tensor_copy(y16[:], k_ps[:])
            nc.scalar.activation(shA[:, 512:1024], k_ps[:], AF.Sin,
                                 bias=cst[:, 0:1], scale=WA / 2)
            nc.scalar.activation(chA[:, 512:1024], k_ps[:], AF.Sin,
                                 bias=cst[:, 1:2], scale=WA / 2)
            nc.scalar.activation(qhB[:, 512:1024], k_ps[:], AF.Sin,
                                 bias=cst[:, 2:3], scale=WB / 4)
            nc.scalar.activation(shB[:, 512:1024], k_ps[:], AF.Sin,
                                 bias=cst[:, 3:4], scale=WB / 2)

            # rk[k] = alpha * (w @ y_proj) as [128k, 1] per k-group
            rk_ps = psA.tile([128, 4], F32, tag="psA")
            for g in range(4):
                nc.tensor.matmul(rk_ps[:, g:g + 1],
                                 y16[:, g * 128:(g + 1) * 128],
                                 cst16[:, 1:2], start=True, stop=True,
                                 skip_group_check=True)
            nc.vector.tensor_copy(rk_sb[:], rk_ps[:])

            # ---- q projection + x seeds ----
            q_ps = psA.tile([128, TQ], F32, tag="psA")
            for c in range(DQ // 128):
                nc.tensor.matmul(q_ps[:], WqT_sb[:, c, :], qT_sb[:, c, :],
                                 start=(c == 0), stop=(c == DQ // 128 - 1))
            nc.scalar.activation(shA[:, 0:512], q_ps[:], AF.Sin,
                                 bias=0.0, scale=WA / 2)
            nc.scalar.activation(chA[:, 0:512], q_ps[:], AF.Sin,
                                 bias=cst[:, 4:5], scale=WA / 2)
            nc.scalar.activation(qhB[:, 0:512], q_ps[:], AF.Sin,
                                 bias=0.0, scale=WB / 4)
            nc.scalar.activation(shB[:, 0:512], q_ps[:], AF.Sin,
                                 bias=0.0, scale=WB / 2)

            # ---- level-1 construction (per x/y half to start early) ----
            # x-half = seed cols 0:512, y-half = seed cols 512:1024
            SXH, SYH = slice(0, 512), slice(512, 1024)

            def build_lev1(sl, ssl, csl):
                # sl: seed slice; ssl/csl: dest slices in the level tiles
                # family A
                nc.vector.scalar_tensor_tensor(
                    lev["a1"][:, ssl], shA[:, sl], 2.0, chA[:, sl],
                    ALU.mult, ALU.mult)
                uA = tmp_pool.tile([128, 512], FP16)
                nc.vector.tensor_tensor(uA[:], shA[:, sl], shA[:, sl], ALU.mult)
                nc.vector.tensor_scalar(
                    lev["a1"][:, csl], uA[:], -2.0, 1.0, ALU.mult, ALU.add)

            def build_lev1B(sl, ssl, csl):
                uB = tmp_pool.tile([128, 512], FP16)
                nc.vector.tensor_tensor(uB[:], qhB[:, sl], qhB[:, sl], ALU.mult)
                nc.vector.tensor_scalar(
                    chB[:, sl], uB[:], -2.0, 1.0, ALU.mult, ALU.add)
                nc.vector.scalar_tensor_tensor(
                    lev["b1"][:, ssl], shB[:, sl], 2.0, chB[:, sl],
                    ALU.mult, ALU.mult)
                uB2 = tmp_pool.tile([128, 512], FP16)
                nc.vector.tensor_tensor(uB2[:], shB[:, sl], shB[:, sl],
                                        ALU.mult)
                nc.vector.tensor_scalar(
                    lev["b1"][:, csl], uB2[:], -2.0, 1.0, ALU.mult, ALU.add)

            def dbl(dst, src):
                # s2 = 2 s c ; c2 = 1 - 2 s^2   (on halves [128,1024])
                nc.vector.scalar_tensor_tensor(
                    dst[:, SH], src[:, SH], 2.0, src[:, CH],
                    ALU.mult, ALU.mult)
                u = tmp_pool.tile([128, 1024], FP16)
                nc.vector.tensor_tensor(u[:], src[:, SH], src[:, SH], ALU.mult)
                nc.vector.tensor_scalar(
                    dst[:, CH], u[:], -2.0, 1.0, ALU.mult, ALU.add)

            def scy_scalar(h):
                s = scy_pool.tile([128, 1024], FP16)
                nc.scalar.activation(s[:, 0:512], lev[h][:, SY], AF.Copy,
                                     bias=0.0, scale=wc[:, HKEYS.index(h):HKEYS.index(h) + 1])
                nc.scalar.activation(s[:, 512:1024], lev[h][:, CY], AF.Copy,
                                     bias=0.0, scale=wc[:, HKEYS.index(h):HKEYS.index(h) + 1])
                return s

            def scy_vector(h):
                s = scy_pool.tile([128, 1024], FP16)
                hi = HKEYS.index(h)
                nc.vector.tensor_scalar_mul(s[:, 0:512], lev[h][:, SY],
                                            wc[:, hi:hi + 1])
                nc.vector.tensor_scalar_mul(s[:, 512:1024], lev[h][:, CY],
                                            wc[:, hi:hi + 1])
                return s

            # psA rotation: k_ps, rk, q_ps, dum -> then plogs reuse those
            # banks (each waits only for the early tile's readers)
            dum_ps = psA.tile([128, 64], F32, tag="psA")
            plogs = [psA.tile([128, TQ], F32, name="plog", tag="psA") for _ in range(4)]

            def products(h, scy_t, first=False, last=False):
                for g in range(4):
                    nc.tensor.matmul(plogs[g][:], scy_t[:, g * 128:(g + 1) * 128],
                                     lev[h][:, CX], start=first, stop=False)
                    nc.tensor.matmul(plogs[g][:],
                                     scy_t[:, 512 + g * 128:512 + (g + 1) * 128],
                                     lev[h][:, SX], start=False, stop=last)

            # y halves first (k path lands first)
            build_lev1(SYH, SY, CY)
            # dummy warm matmul to bridge the PE gap during seed generation
            nc.tensor.matmul(dum_ps[:], lev["a1"][:, 512:640],
                             lev["a1"][:, 512:576], start=True, stop=True,
                             skip_group_check=True)
            build_lev1(SXH, SX, CX)

            s_a1 = scy_scalar("a1")
            products("a1", s_a1, first=True)

            dbl(lev["a2"], lev["a1"])
            s_a2 = scy_scalar("a2")
            products("a2", s_a2)

            build_lev1B(SYH, SY, CY)
            build_lev1B(SXH, SX, CX)
            s_b1 = scy_scalar("b1")
            products("b1", s_b1)

            # mulB = [2 c1b | 2 c1b]
            nc.vector.tensor_scalar(mulB[:, SH], lev["b1"][:, CH], 2.0, None,
                                    ALU.mult)
            nc.vector.tensor_scalar(mulB[:, CH], lev["b1"][:, CH], 2.0, None,
                                    ALU.mult)

            dbl(lev["b2"], lev["b1"])
            s_b2 = scy_scalar("b2")
            products("b2", s_b2)

            dbl(lev["a4"], lev["a2"])
            s_a4 = scy_scalar("a4")
            products("a4", s_a4)

            # b3 = mulB * b2 - b1   (Chebyshev step, full tile)
            t3 = t2k_pool.tile([128, 2048], FP16)
            nc.vector.tensor_tensor(t3[:], mulB[:], lev["b2"][:], ALU.mult)
            nc.vector.tensor_tensor(lev["b3"][:], t3[:], lev["b1"][:],
                                    ALU.subtract)
            s_b3 = scy_scalar("b3")
            products("b3", s_b3)

            dbl(lev["a8"], lev["a4"])
            s_a8 = scy_vector("a8")
            products("a8", s_a8)

            dbl(lev["b4"], lev["b2"])
            s_b4 = scy_vector("b4")
            products("b4", s_b4, last=True)

            # ---- epilogue ----
            # exp tiles [128k, TQ] fp16, rk bias folded into Exp
            for g in range(4):
                nc.scalar.activation(exp_t[g][:], plogs[g][:], AF.Exp,
                                     bias=rk_sb[:, g:g + 1], scale=1.0)

            # out[q, d] = sum_k exp[k, q] value[k, d]; rowsums via ones column
            out_ps = [pout_pool.tile([128, DV], F32, name="out_ps", tag="out")
                      for _ in range(4)]
            rsq_ps = psA.tile([128, 4], F32, tag="psA")
            for gk in range(4):
                for gq in range(4):
                    nc.tensor.matmul(rsq_ps[:, gq:gq + 1],
                                     exp_t[gk][:, gq * 128:(gq + 1) * 128],
                                     cst16[:, 0:1],
                                     start=(gk == 0 and gq == 0),
                                     stop=(gk == 3 and gq == 3),
                                     skip_group_check=True)
                for gq in range(4):
                    nc.tensor.matmul(out_ps[gq][:],
                                     exp_t[gk][:, gq * 128:(gq + 1) * 128],
                                     value_sb[:, gk, :],
                                     start=(gk == 0), stop=(gk == 3))

            # reciprocal of rowsums (q on partitions, for out normalization)
            recq = epi_pool.tile([128, 4], F32)
            nc.vector.reciprocal(recq[:, :], rsq_ps[:, :])
            # rowsums as a single row [1, TQ]: transpose + partition gather
            rsq16 = epi_pool.tile([128, 4], FP16)
            nc.vector.tensor_copy(rsq16[:], rsq_ps[:])
            rsqT_ps = psA.tile([4, 128], FP16, tag="psA")
            nc.tensor.transpose(rsqT_ps[:], rsq16[:], ident[:])
            rsqT = epi_pool.tile([4, 128], FP16)
            nc.vector.tensor_copy(rsqT[:], rsqT_ps[:])
            rs_row = pers.tile([1, TQ], FP16, tag="rs_row")
            for g in range(4):
                eng = (nc.sync, nc.gpsimd, nc.sync, nc.gpsimd)[g]
                eng.dma_start(out=rs_row[0:1, g * 128:(g + 1) * 128],
                              in_=rsqT[g:g + 1, :])
            # broadcast across partitions (rank-1 matmul), then reciprocal
            rsB_ps = psA.tile([128, TQ], F32, tag="psA")
            nc.tensor.matmul(rsB_ps[:], onesR[:, :], rs_row[0:1, :],
                             start=True, stop=True)
            recB32 = epi_pool.tile([128, TQ], F32)
            nc.vector.reciprocal(recB32[:, :], rsB_ps[:, :])
            recB = epi_pool.tile([128, TQ], FP16)
            nc.vector.tensor_copy(recB[:], recB32[:])

            # attn tiles (normalized, fp16) + DMA out
            for g in range(4):
                at = epi_pool.tile([128, TQ], FP16)
                nc.vector.tensor_tensor(at[:], exp_t[g][:], recB[:], ALU.mult)
                eng = nc.sync if g % 2 == 0 else nc.gpsimd
                eng.dma_start(out=attnT_o[g * 128:(g + 1) * 128, :], in_=at[:])

            # out tiles: normalize via ACT scale (2 on DVE ts, 2 on ScalarE)
            for gq in range(4):
                ot = epi_pool.tile([128, DV], FP16)
                if gq % 2 == 0:
                    nc.vector.tensor_scalar(ot[:], out_ps[gq][:],
                                            recq[:, gq:gq + 1], None, ALU.mult)
                else:
                    nc.scalar.activation(ot[:], out_ps[gq][:], AF.Copy,
                                         bias=0.0, scale=recq[:, gq:gq + 1])
                eng = nc.gpsimd if gq % 2 == 0 else nc.sync
                eng.dma_start(out=outN_o[gq * 128:(gq + 1) * 128, 0:256],
                              in_=ot[:, 0:256])
                eng2 = nc.sync if gq % 2 == 0 else nc.gpsimd
                eng2.dma_start(out=outN_o[gq * 128:(gq + 1) * 128, 256:512],
                               in_=ot[:, 256:512])

    nc.compile()
    return nc


def _get_nc():
    if "nc" not in _CACHE:
        _CACHE["nc"] = build_nc()
    return _CACHE["nc"]


def make_in_maps(query, key, value, Wq, Wk, bias, w_w, **_):
    coeffs, alpha = _fit_params()
    w = np.asarray(w_w, dtype=np.float64).reshape(A)
    b = np.asarray(bias, dtype=np.float64).reshape(A)
    WqT = np.ascontiguousarray(Wq.T).astype(np.float16)
    WkT = np.ascontiguousarray(Wk.T).astype(np.float16)
    cst = np.zeros((A, 8), dtype=np.float32)
    cst[:, 0] = WA / 2 * b
    cst[:, 1] = WA / 2 * b + HALF_PI
    cst[:, 2] = WB / 4 * b
    cst[:, 3] = WB / 2 * b
    cst[:, 4] = HALF_PI
    cst16 = np.zeros((A, 2), dtype=np.float16)
    cst16[:, 0] = 1.0
    cst16[:, 1] = (alpha * w).astype(np.float16)
    wc = (w[:, None] * coeffs[None, :]).astype(np.float32)  # [A, 8]
    onesR = np.ones((1, 128), dtype=np.float16)
    ident = np.eye(128, dtype=np.float16)
    in_maps = []
    for bb in range(B):
        in_maps.append({
            "qT": np.ascontiguousarray(query[bb].T).astype(np.float16),
            "kT": np.ascontiguousarray(key[bb].T).astype(np.float16),
            "value": np.ascontiguousarray(value[bb]).astype(np.float16),
            "WqT": WqT,
            "WkT": WkT,
            "cst": cst,
            "cst16": cst16,
            "wc": wc,
            "onesR": onesR,
            "ident": ident,
        })
    return in_maps


def run(inputs, trace=False, **kwargs):
    nc = _get_nc()
    in_maps = make_in_maps(**{k: np.asarray(v) for k, v in inputs.items()})
    res = run_bass_kernel_spmd(
        nc, in_maps, list(range(N_CORES)), trace=trace, **kwargs
    )
    output = np.stack([res.results[bb]["outN"].astype(np.float32)
                       for bb in range(B)])
    attn = np.stack([
        np.ascontiguousarray(res.results[bb]["attnT"].T).astype(np.float32)
        for bb in range(B)])
    return (output, attn), res


def kernel(**inputs):
    (output, attn), _ = run(inputs)
    return output, attn


# revision 14
# speedup vs baseline: 1.2378x; 1.1375x over previous
"""Additive (Bahdanau) attention on 8 trn2 NeuronCores — flipped sine-expansion.

Math per batch element b (one core each):
  logits[q,k] = sum_a w_a * tanh(x_qa + y_ka),  x = query@Wq^T, y = key@Wk^T + bias
  attn = softmax_k(logits);  out = attn @ value

tanh(z) ~ alpha*z + sum_h c_h sin(w_h z) with frequency set
  WA*{1,2,4,8} u WB*{1,2,3,4}  (fit on [-8.6, 8.6]).
Each sine factors sin(w(x+y)) = sin(wx)cos(wy)+cos(wx)sin(wy), so logits are
16 rank-128 fp16 matmuls accumulated in PSUM.  Logits are computed
TRANSPOSED ([k, q] with k on PSUM partitions):
  - the linear-in-y term alpha*(w@Wk)@kT becomes a per-partition Exp bias
    (no rank-1 PSUM opens); the per-q linear term cancels in softmax
  - no PE transposes in the out = attn@value epilogue (exp tiles are
    directly the stationary operand)
  - attn is written transposed as fp16; host transposes it back

Harmonic ladder in fp16 on the DVE.  Sine parts are stored DOWN-SCALED,
sigma_m = s_m / k_m, so the double-angle step s2=2*s*c becomes a plain
tensor_tensor (sigma2 = sigma*c), which runs in 2x packed mode — the
scalar_tensor_tensor form only has 1x uops.  The k_m factors fold into the
per-harmonic product scale w_a*c_h*k_h applied to the y-side slices
(scaled copies split between ScalarE and DVE).  b3 uses one Chebyshev step
with multiplier [4c1|2c1].  Seeds are ACT Sin at half/quarter angle.
"""

import numpy as np

import concourse.bass as bass
import concourse.tile as tile
from concourse import bacc, mybir
from concourse.bass_utils import run_bass_kernel_spmd

F32 = mybir.dt.float32
FP16 = mybir.dt.float16
AF = mybir.ActivationFunctionType
ALU = mybir.AluOpType

B, TQ, TK, DQ, DK, DV, A = 8, 512, 512, 512, 512, 512, 128
N_CORES = 8

WA, WB = 0.36, 0.55
RFIT = 8.6
HKEYS = ["a1", "a2", "a4", "a8", "b1", "b2", "b3", "b4"]
FREQS = [WA, 2 * WA, 4 * WA, 8 * WA, WB, 2 * WB, 3 * WB, 4 * WB]
# sigma_m = s_m / KS[m]; c parts are stored true
KS = [2, 4, 8, 16, 2, 4, 2, 8]
HALF_PI = float(np.pi / 2)

# quadrant slices of a level tile [sig_x | sig_y | c_x | c_y]
SX, SY, CX, CY = (slice(0, 512), slice(512, 1024),
                  slice(1024, 1536), slice(1536, 2048))
SH, CH = slice(0, 1024), slice(1024, 2048)  # sigma-half, c-half

_CACHE = {}


def _fit_params():
    if "fit" in _CACHE:
        return _CACHE["fit"]
    zg = np.linspace(-RFIT, RFIT, 6001)
    t = np.tanh(zg)
    Amat = np.stack([np.sin(w * zg) for w in FREQS] + [zg], 1)
    coef, *_ = np.linalg.lstsq(Amat, t, rcond=None)
    _CACHE["fit"] = (coef[:-1].astype(np.float64), float(coef[-1]))
    return _CACHE["fit"]


def build_nc():
    nc = bacc.Bacc(None, target_bir_lowering=False, debug=False)

    qT = nc.declare_dram_parameter("qT", [DQ, TQ], FP16, isOutput=False)
    kT = nc.declare_dram_parameter("kT", [DK, TK], FP16, isOutput=False)
    val = nc.declare_dram_parameter("value", [TK, DV], FP16, isOutput=False)
    WqT = nc.declare_dram_parameter("WqT", [DQ, A], FP16, isOutput=False)
    WkT = nc.declare_dram_parameter("WkT", [DK, A], FP16, isOutput=False)
    # f32 consts: c0=WA/2*b, c1=WA/2*b+pi/2, c2=WB/4*b, c3=WB/2*b, c4=pi/2
    cst_d = nc.declare_dram_parameter("cst", [A, 8], F32, isOutput=False)
    # fp16 consts: c0=1.0, c1=alpha*w_a
    cst16_d = nc.declare_dram_parameter("cst16", [A, 2], FP16, isOutput=False)
    wc_d = nc.declare_dram_parameter("wc", [A, 8], F32, isOutput=False)
    onesR_d = nc.declare_dram_parameter("onesR", [1, 128], FP16, isOutput=False)
    ident_d = nc.declare_dram_parameter("ident", [128, 128], FP16,
                                        isOutput=False)
    attnT_o = nc.declare_dram_parameter("attnT", [TK, TQ], FP16, isOutput=True)
    outN_o = nc.declare_dram_parameter("outN", [TQ, DV], FP16, isOutput=True)

    with tile.TileContext(nc) as tc:
        with (
            tc.tile_pool(name="pers", bufs=1) as pers,
            tc.tile_pool(name="tmp", bufs=3) as tmp_pool,
            tc.tile_pool(name="t2k", bufs=1) as t2k_pool,
            tc.tile_pool(name="scy", bufs=3) as scy_pool,
            tc.tile_pool(name="epi", bufs=2) as epi_pool,
            tc.tile_pool(name="psA", bufs=4, space="PSUM") as psA,
            tc.tile_pool(name="pout", bufs=4, space="PSUM") as pout_pool,
        ):
            # ---- persistent tiles ----
            cst = pers.tile([128, 8], F32, tag="cst")
            cst16 = pers.tile([128, 2], FP16, tag="cst16")
            wc = pers.tile([128, 8], F32, tag="wc")
            onesR = pers.tile([1, 128], FP16, tag="onesR")
            ident = pers.tile([128, 128], FP16, tag="ident")
            WkT_sb = pers.tile([128, DK // 128, A], FP16, tag="WkT_sb")
            WqT_sb = pers.tile([128, DQ // 128, A], FP16, tag="WqT_sb")
            kT_sb = pers.tile([128, DK // 128, TK], FP16, tag="kT_sb")
            qT_sb = pers.tile([128, DQ // 128, TQ], FP16, tag="qT_sb")
            value_sb = pers.tile([128, TK // 128, DV], FP16, tag="value_sb")
            y16 = pers.tile([128, TK], FP16, tag="y16")
            rk_sb = pers.tile([128, 4], F32, tag="rk_sb")
            # seeds [x | y]
            shA = pers.tile([128, 1024], FP16, tag="shA")
            chA = pers.tile([128, 1024], FP16, tag="chA")
            qhB = pers.tile([128, 1024], FP16, tag="qhB")
            shB = pers.tile([128, 1024], FP16, tag="shB")
            chB = pers.tile([128, 1024], FP16, tag="chB")
            lev = {h: pers.tile([128, 2048], FP16, name=f"lev_{h}",
                                tag=f"lev_{h}")
                   for h in HKEYS}
            mulB = pers.tile([128, 2048], FP16, tag="mulB")
            exp_t = [pers.tile([128, TQ], FP16, name=f"exp{g}", tag=f"exp{g}")
                     for g in range(4)]
            rec_row = pers.tile([1, TQ], FP16, tag="rec_row")

            # ---- input DMA (small chunks spread over queues/sequencers) ----
            kT_re = kT.rearrange("(c p) t -> p c t", p=128)
            qT_re = qT.rearrange("(c p) t -> p c t", p=128)
            val_re = val.rearrange("(c p) d -> p c d", p=128)
            WkT_re = WkT.rearrange("(c p) a -> p c a", p=128)
            WqT_re = WqT.rearrange("(c p) a -> p c a", p=128)
            # sync: consts, WkT chunks, kT left halves
            nc.sync.dma_start(out=cst[:], in_=cst_d[:, :])
            for c in range(4):
                nc.sync.dma_start(out=WkT_sb[:, c:c + 1, :],
                                  in_=WkT_re[:, c:c + 1, :])
                nc.sync.dma_start(out=kT_sb[:, c:c + 1, 0:256],
                                  in_=kT_re[:, c:c + 1, 0:256])
            nc.sync.dma_start(out=wc[:], in_=wc_d[:, :])
            # scalar: fp16 consts, kT right halves
            nc.scalar.dma_start(out=cst16[:], in_=cst16_d[:, :])
            for c in range(4):
                nc.scalar.dma_start(out=kT_sb[:, c:c + 1, 256:512],
                                    in_=kT_re[:, c:c + 1, 256:512])
            # gpsimd: WqT chunks, qT halves, then low-priority tensors
            for c in range(4):
                nc.gpsimd.dma_start(out=WqT_sb[:, c:c + 1, :],
                                    in_=WqT_re[:, c:c + 1, :])
                nc.gpsimd.dma_start(out=qT_sb[:, c:c + 1, 0:256],
                                    in_=qT_re[:, c:c + 1, 0:256])
                nc.gpsimd.dma_start(out=qT_sb[:, c:c + 1, 256:512],
                                    in_=qT_re[:, c:c + 1, 256:512])
            nc.gpsimd.dma_start(out=onesR[:], in_=onesR_d[:, :])
            nc.gpsimd.dma_start(out=ident[:], in_=ident_d[:, :])
            for c in range(4):
                eng = nc.scalar if c % 2 == 0 else nc.gpsimd
                eng.dma_start(out=value_sb[:, c:c + 1, :],
                              in_=val_re[:, c:c + 1, :])

            # ---- k projection + y seeds ----
            k_ps = psA.tile([128, TK], F32, tag="psA")
            for c in range(DK // 128):
                nc.tensor.matmul(k_ps[:], WkT_sb[:, c, :], kT_sb[:, c, :],
                                 start=(c == 0), stop=(c == DK // 128 - 1))
            nc.vector.tensor_copy(y16[:], k_ps[:])
            nc.scalar.activation(shA[:, 512:1024], k_ps[:], AF.Sin,
                                 bias=cst[:, 0:1], scale=WA / 2)
            nc.scalar.activation(chA[:, 512:1024], k_ps[:], AF.Sin,
                                 bias=cst[:, 1:2], scale=WA / 2)
            nc.scalar.activation(qhB[:, 512:1024], k_ps[:], AF.Sin,
                                 bias=cst[:, 2:3], scale=WB / 4)
            nc.scalar.activation(shB[:, 512:1024], k_ps[:], AF.Sin,
                                 bias=cst[:, 3:4], scale=WB / 2)

            # rk[k] = alpha * (w @ y_proj) as [128k, 1] per k-group
            rk_ps = psA.tile([128, 4], F32, tag="psA")
            for g in range(4):
                nc.tensor.matmul(rk_ps[:, g:g + 1],
                                 y16[:, g * 128:(g + 1) * 128],
                                 cst16[:, 1:2], start=True, stop=True,
                                 skip_group_check=True)
            nc.vector.tensor_copy(rk_sb[:], rk_ps[:])

            # ---- q projection + x seeds (A-family seeds first) ----
            q_ps = psA.tile([128, TQ], F32, tag="psA")
            for c in range(DQ // 128):
                nc.tensor.matmul(q_ps[:], WqT_sb[:, c, :], qT_sb[:, c, :],
                                 start=(c == 0), stop=(c == DQ // 128 - 1))
            nc.scalar.activation(shA[:, 0:512], q_ps[:], AF.Sin,
                                 bias=0.0, scale=WA / 2)
            nc.scalar.activation(chA[:, 0:512], q_ps[:], AF.Sin,
                                 bias=cst[:, 4:5], scale=WA / 2)
            nc.scalar.activation(qhB[:, 0:512], q_ps[:], AF.Sin,
                                 bias=0.0, scale=WB / 4)
            nc.scalar.activation(shB[:, 0:512], q_ps[:], AF.Sin,
                                 bias=0.0, scale=WB / 2)

            # dense dummy matmuls to keep the PE busy (HAM warm) while
            # seeds/levels are generated
            dum_ps = psA.tile([128, 64], F32, tag="psA")
            for i in range(6):
                nc.tensor.matmul(dum_ps[:], WkT_sb[:, i % 4, :],
                                 WkT_sb[:, (i + 1) % 4, 0:64],
                                 start=True, stop=True, skip_group_check=True)

            plogs = [psA.tile([128, TQ], F32, name="plog", tag="psA")
                     for _ in range(4)]

            SXH, SYH = slice(0, 512), slice(512, 1024)

            def build_lev1(sl, ssl, csl):
                # family A: sigma1 = shA*chA (k=2); c1 = 1 - 2 shA^2
                nc.vector.tensor_tensor(lev["a1"][:, ssl], shA[:, sl],
                                        chA[:, sl], ALU.mult)
                uA = tmp_pool.tile([128, 512], FP16, tag="tmp")
                nc.vector.tensor_tensor(uA[:], shA[:, sl], shA[:, sl], ALU.mult)
                nc.vector.tensor_scalar(
                    lev["a1"][:, csl], uA[:], -2.0, 1.0, ALU.mult, ALU.add)

            def build_lev1B(sl, ssl, csl):
                uB = tmp_pool.tile([128, 512], FP16, tag="tmp")
                nc.vector.tensor_tensor(uB[:], qhB[:, sl], qhB[:, sl], ALU.mult)
                nc.vector.tensor_scalar(
                    chB[:, sl], uB[:], -2.0, 1.0, ALU.mult, ALU.add)
                nc.vector.tensor_tensor(lev["b1"][:, ssl], shB[:, sl],
                                        chB[:, sl], ALU.mult)
                uB2 = tmp_pool.tile([128, 512], FP16, tag="tmp")
                nc.vector.tensor_tensor(uB2[:], shB[:, sl], shB[:, sl],
                                        ALU.mult)
                nc.vector.tensor_scalar(
                    lev["b1"][:, csl], uB2[:], -2.0, 1.0, ALU.mult, ALU.add)

            def dbl(dst, src, ksrc):
                # sigma_2m = sigma_m * c_m ; c_2m = 1 - 2 k^2 sigma_m^2
                nc.vector.tensor_tensor(dst[:, SH], src[:, SH], src[:, CH],
                                        ALU.mult)
                u = tmp_pool.tile([128, 1024], FP16, tag="tmp")
                nc.vector.tensor_tensor(u[:], src[:, SH], src[:, SH], ALU.mult)
                nc.vector.tensor_scalar(
                    dst[:, CH], u[:], -2.0 * ksrc * ksrc, 1.0,
                    ALU.mult, ALU.add)

            def scy_make(h, s_eng, c_eng):
                s = scy_pool.tile([128, 1024], FP16, name=f"scy_{h}", tag="scy")
                hi = HKEYS.index(h)
                for half, src_sl, eng in ((slice(0, 512), SY, s_eng),
                                          (slice(512, 1024), CY, c_eng)):
                    if eng == "S":
                        nc.scalar.activation(s[:, half], lev[h][:, src_sl],
                                             AF.Copy, bias=0.0,
                                             scale=wc[:, hi:hi + 1])
                    else:
                        nc.vector.tensor_scalar_mul(s[:, half],
                                                    lev[h][:, src_sl],
                                                    wc[:, hi:hi + 1])
                return s

            def products(h, scy_t, first=False, last=False):
                for g in range(4):
                    nc.tensor.matmul(plogs[g][:],
                                     scy_t[:, g * 128:(g + 1) * 128],
                                     lev[h][:, CX], start=first, stop=False)
                    nc.tensor.matmul(plogs[g][:],
                                     scy_t[:, 512 + g * 128:512 + (g + 1) * 128],
                                     lev[h][:, SX], start=False, stop=last)

            # ---- ladder + products, interleaved ----
            build_lev1(SYH, SY, CY)
            build_lev1(SXH, SX, CX)
            s_a1 = scy_make("a1", "S", "V")
            products("a1", s_a1, first=True)

            dbl(lev["a2"], lev["a1"], 2)
            s_a2 = scy_make("a2", "S", "V")
            products("a2", s_a2)

            build_lev1B(SYH, SY, CY)
            build_lev1B(SXH, SX, CX)
            s_b1 = scy_make("b1", "S", "V")
            products("b1", s_b1)

            # mulB = [4 c1b | 2 c1b] (sigma-half needs k2b/k3b * 2 = 4)
            nc.vector.tensor_scalar_mul(mulB[:, SH], lev["b1"][:, CH], 4.0)
            nc.vector.tensor_scalar_mul(mulB[:, CH], lev["b1"][:, CH], 2.0)

            dbl(lev["b2"], lev["b1"], 2)
            s_b2 = scy_make("b2", "S", "V")
            products("b2", s_b2)

            dbl(lev["a4"], lev["a2"], 4)
            s_a4 = scy_make("a4", "S", "V")
            products("a4", s_a4)

            # b3: sigma3 = 4 c1b sigma2 - sigma1 ; c3 = 2 c1b c2 - c1
            t3 = t2k_pool.tile([128, 2048], FP16, tag="t3")
            nc.vector.tensor_tensor(t3[:], mulB[:], lev["b2"][:], ALU.mult)
            nc.vector.tensor_tensor(lev["b3"][:], t3[:], lev["b1"][:],
                                    ALU.subtract)
            s_b3 = scy_make("b3", "S", "S")
            products("b3", s_b3)

            dbl(lev["a8"], lev["a4"], 8)
            s_a8 = scy_make("a8", "S", "V")
            products("a8", s_a8)

            dbl(lev["b4"], lev["b2"], 4)
            s_b4 = scy_make("b4", "S", "V")
            products("b4", s_b4, last=True)

            # ---- epilogue ----
            for g in range(4):
                nc.scalar.activation(exp_t[g][:], plogs[g][:], AF.Exp,
                                     bias=rk_sb[:, g:g + 1], scale=1.0)

            # out[q, d] = sum_k exp[k, q] value[k, d]; rowsums via ones column
            out_ps = [pout_pool.tile([128, DV], F32, name="out_ps", tag="out")
                      for _ in range(4)]
            rsq_ps = psA.tile([128, 4], F32, tag="psA")
            for gk in range(4):
                for gq in range(4):
                    nc.tensor.matmul(rsq_ps[:, gq:gq + 1],
                                     exp_t[gk][:, gq * 128:(gq + 1) * 128],
                                     cst16[:, 0:1],
                                     start=(gk == 0 and gq == 0),
                                     stop=(gk == 3 and gq == 3),
                                     skip_group_check=True)
                for gq in range(4):
                    nc.tensor.matmul(out_ps[gq][:],
                                     exp_t[gk][:, gq * 128:(gq + 1) * 128],
                                     value_sb[:, gk, :],
                                     start=(gk == 0), stop=(gk == 3))

            # one small reciprocal; broadcast it to a [128, TQ] tile
            recq = epi_pool.tile([128, 4], F32, tag="rec4")
            nc.vector.reciprocal(recq[:, :], rsq_ps[:, :])
            recq16 = epi_pool.tile([128, 4], FP16, tag="rec4")
            nc.vector.tensor_copy(recq16[:], recq[:])
            recT_ps = psA.tile([4, 128], FP16, tag="psA")
            nc.tensor.transpose(recT_ps[:], recq16[:], ident[:])
            recT = epi_pool.tile([4, 128], FP16, tag="rec4")
            nc.vector.tensor_copy(recT[:], recT_ps[:])
            for g in range(4):
                eng = (nc.sync, nc.gpsimd, nc.sync, nc.gpsimd)[g]
                eng.dma_start(out=rec_row[0:1, g * 128:(g + 1) * 128],
                              in_=recT[g:g + 1, :])
            recB_ps = psA.tile([128, TQ], F32, tag="psA")
            nc.tensor.matmul(recB_ps[:], onesR[:, :], rec_row[0:1, :],
                             start=True, stop=True)
            recB = epi_pool.tile([128, TQ], FP16, tag="recB")
            nc.vector.tensor_copy(recB[:], recB_ps[:])

            # attn tiles (normalized, fp16) + DMA out in halves
            for g in range(4):
                at = epi_pool.tile([128, TQ], FP16, name=f"at{g}", tag="at")
                nc.vector.tensor_tensor(at[:], exp_t[g][:], recB[:], ALU.mult)
                eng = nc.sync if g % 2 == 0 else nc.gpsimd
                eng.dma_start(out=attnT_o[g * 128:(g + 1) * 128, 0:256],
                              in_=at[:, 0:256])
                eng2 = nc.gpsimd if g % 2 == 0 else nc.sync
                eng2.dma_start(out=attnT_o[g * 128:(g + 1) * 128, 256:512],
                               in_=at[:, 256:512])

            # out tiles: normalize via per-partition reciprocal scale
            for gq in range(4):
                ot = epi_pool.tile([128, DV], FP16, name=f"ot{gq}", tag="ot")
                if gq % 2 == 0:
                    nc.vector.tensor_scalar_mul(ot[:], out_ps[gq][:],
                                                recq[:, gq:gq + 1])
                else:
                    nc.scalar.activation(ot[:], out_ps[gq][:], AF.Copy,
                                         bias=0.0, scale=recq[:, gq:gq + 1])
                eng = nc.gpsimd if gq % 2 == 0 else nc.sync
                eng.dma_start(out=outN_o[gq * 128:(gq + 1) * 128, 0:256],
                              in_=ot[:, 0:256])
                eng2 = nc.sync if gq % 2 == 0 else nc.gpsimd
                eng2.dma_start(out=outN_o[gq * 128:(gq + 1) * 128, 256:512],
                               in_=ot[:, 256:512])

    nc.compile()
    return nc


def _get_nc():
    if "nc" not in _CACHE:
        _CACHE["nc"] = build_nc()
    return _CACHE["nc"]


def make_in_maps(query, key, value, Wq, Wk, bias, w_w, **_):
    coeffs, alpha = _fit_params()
    w = np.asarray(w_w, dtype=np.float64).reshape(A)
    b = np.asarray(bias, dtype=np.float64).reshape(A)
    WqT = np.ascontiguousarray(Wq.T).astype(np.float16)
    WkT = np.ascontiguousarray(Wk.T).astype(np.float16)
    cst = np.zeros((A, 8), dtype=np.float32)
    cst[:, 0] = WA / 2 * b
    cst[:, 1] = WA / 2 * b + HALF_PI
    cst[:, 2] = WB / 4 * b
    cst[:, 3] = WB / 2 * b
    cst[:, 4] = HALF_PI
    cst16 = np.zeros((A, 2), dtype=np.float16)
    cst16[:, 0] = 1.0
    cst16[:, 1] = (alpha * w).astype(np.float16)
    # per-harmonic product scale, with the sigma ladder k_h folded in
    wc = (w[:, None] * (coeffs * np.array(KS, np.float64))[None, :]) \
        .astype(np.float32)
    onesR = np.ones((1, 128), dtype=np.float16)
    ident = np.eye(128, dtype=np.float16)
    in_maps = []
    for bb in range(B):
        in_maps.append({
            "qT": np.ascontiguousarray(query[bb].T).astype(np.float16),
            "kT": np.ascontiguousarray(key[bb].T).astype(np.float16),
            "value": np.ascontiguousarray(value[bb]).astype(np.float16),
            "WqT": WqT,
            "WkT": WkT,
            "cst": cst,
            "cst16": cst16,
            "wc": wc,
            "onesR": onesR,
            "ident": ident,
        })
    return in_maps


def run(inputs, trace=False, **kwargs):
    nc = _get_nc()
    in_maps = make_in_maps(**{k: np.asarray(v) for k, v in inputs.items()})
    res = run_bass_kernel_spmd(
        nc, in_maps, list(range(N_CORES)), trace=trace, **kwargs
    )
    output = np.stack([res.results[bb]["outN"].astype(np.float32)
                       for bb in range(B)])
    attn = np.stack([
        np.ascontiguousarray(res.results[bb]["attnT"].T).astype(np.float32)
        for bb in range(B)])
    return (output, attn), res


def kernel(**inputs):
    (output, attn), _ = run(inputs)
    return output, attn
